# revision 12
# baseline (speedup 1.0000x reference)
"""TRN2 Bass kernel for nn_ContrastiveLoss_45277545235064.

Reference computation (see problem):
    f   = features / ||features||        (row-normalize, fp32)
    E   = exp((f @ f.T) / tau)           [N, N], tau = 0.1
    pos = sum_{same group} E - exp(1/tau)
    neg = sum_{other groups} E
    loss = sum(-log(pos / neg)) / N

Sharding: data-parallel over anchors. Each of the 8 cores computes the
[1024, 8192] slab of E for its anchor rows against the full feature set,
reducing each row on the fly (E never materializes in HBM).

Numerics: the main sweep runs in bf16 (full PE speed). The same-group
positives sit in the 128x128 block-diagonal tiles and include the self term
exp(1/tau) ~ 22026 which is subtracted; a bf16 self term would wreck pos (a
0.4% error there is ~880 absolute vs pos ~ 10). So each core recomputes its
8 diagonal 128x128 blocks in fp32 and splices them in:
    R      = bf16 row sum over all 8192 cols (includes bf16 diag block)
    B16    = bf16 diag-block row sum   (bit-identical recompute)
    B32    = fp32 diag-block row sum
    POS    = fp32 masked (same-group, no self) diag-block row sum
    EII    = fp32 self term
    pos    = POS                       (self excluded => exact cancellation)
    neg    = R - B16 + B32 - POS - EII
Host finishes with loss = mean(log(neg) - log(pos)) in float64.

rsqrt is computed with 6 Newton iterations on the vector engine from a fixed
seed (feature rows are ~N(0,1), so ||x||^2 ~ 256); this avoids the scalar
engine's low-precision Sqrt table and any activation-table switch (Exp is
the only ACT function used).
"""

import sys

sys.path.insert(0, "/opt/trn_rl_repo")

import numpy as np

import concourse.bass as bass  # noqa: F401  (import keeps bass registered)
import concourse.mybir as mybir
import concourse.tile as tile
from concourse import bacc
from concourse.bass_utils import run_bass_kernel_spmd

P = 128
N = 8192
D = 256
CORES = 8
SLAB = N // CORES  # 1024 anchor rows per core
TS = SLAB // P  # 8 anchor tiles per core
CT = N // P  # 64 column tiles
NCH = 512  # matmul moving free dim (one PSUM bank)
NCHUNKS = N // NCH  # 16
GROUP = 8
INV_TAU = 10.0

f32 = mybir.dt.float32
bf16 = mybir.dt.bfloat16
OP = mybir.AluOpType
EXP = mybir.ActivationFunctionType.Exp

_cache: dict = {}


def _build(debug: bool = False, stop_after: int = 6):
    nc = bacc.Bacc(
        "TRN2",
        target_bir_lowering=False,
        debug=debug,
        num_devices=CORES,
    )

    feats_d = nc.dram_tensor("feats", [N, D], f32, kind="ExternalInput")
    slab_d = nc.dram_tensor("slab", [SLAB, D], f32, kind="ExternalInput")
    posmask_d = nc.dram_tensor("posmask", [P, P], f32, kind="ExternalInput")
    eye_d = nc.dram_tensor("eyemask", [P, P], f32, kind="ExternalInput")
    pos_d = nc.dram_tensor("pos", [SLAB], f32, kind="ExternalOutput")
    neg_d = nc.dram_tensor("neg", [SLAB], f32, kind="ExternalOutput")

    W = TS + CT  # 72 row tiles to normalize (slab first, then full)

    with tile.TileContext(nc) as tc:
        with (
            tc.tile_pool(name="persist", bufs=1) as pp,
            tc.tile_pool(name="work", bufs=3) as wp,
            tc.tile_pool(name="psum_mm", bufs=4, space="PSUM") as pmm,
            tc.tile_pool(name="psum_tp", bufs=2, space="PSUM") as ptp,
            tc.tile_pool(name="psum_dg", bufs=2, space="PSUM") as pdg,
        ):
            # persistent SBUF tensors
            feats_sb = pp.tile([P, CT, D], f32)
            slab_sb = pp.tile([P, TS, D], f32)
            fTb_0 = pp.tile([P, N], bf16)
            fTb_1 = pp.tile([P, N], bf16)
            fTs16_0 = pp.tile([P, SLAB], bf16)
            fTs16_1 = pp.tile([P, SLAB], bf16)
            fTs32_0 = pp.tile([P, SLAB], f32)
            fTs32_1 = pp.tile([P, SLAB], f32)
            posmask = pp.tile([P, P], f32)
            eye = pp.tile([P, P], f32)
            ssq = pp.tile([P, W], f32)
            rr = pp.tile([P, W], f32)
            nrt = pp.tile([P, W], f32)
            accs = pp.tile([P, TS * NCHUNKS], f32)
            B32 = pp.tile([P, TS], f32)
            B16 = pp.tile([P, TS], f32)
            POS = pp.tile([P, TS], f32)
            EII = pp.tile([P, TS], f32)
            Racc = pp.tile([P, TS], f32)
            NEG = pp.tile([P, TS], f32)

            nc.sync.dma_start(posmask[:], posmask_d[:])
            nc.sync.dma_start(eye[:], eye_d[:])

            # ---- phase 1: load rows + sum of squares (slab tiles first)
            def load_and_ssq(src_ap, dst_slice, col):
                nc.sync.dma_start(dst_slice, src_ap)
                junk = wp.tile([P, D], f32, tag="ssq_junk")
                nc.vector.tensor_mul(junk[:], dst_slice, dst_slice)
                nc.vector.tensor_reduce(
                    out=ssq[:, col : col + 1],
                    in_=junk[:],
                    axis=mybir.AxisListType.X,
                    op=OP.add,
                )

            for j in range(TS):
                load_and_ssq(slab_d[j * P : (j + 1) * P, :], slab_sb[:, j, :], j)
            for t in range(CT):
                load_and_ssq(feats_d[t * P : (t + 1) * P, :], feats_sb[:, t, :], TS + t)

            def _finish_early():
                nc.vector.memset(POS[:], 1.0)
                nc.vector.memset(NEG[:], 1.0)
                nc.sync.dma_start(pos_d.ap().rearrange("(m p) -> p m", p=P), POS[:])
                nc.sync.dma_start(neg_d.ap().rearrange("(m p) -> p m", p=P), NEG[:])

            # ---- phase 2: rsqrt via Newton iterations (DVE only), batched so
            # transposes can start before all loads finish.
            if stop_after >= 2:
                nc.vector.memset(rr[:], 0.0625)
                batches = [(0, 24), (24, 48), (48, W)]
                for b0, b1 in batches:
                    for _ in range(6):
                        nc.vector.tensor_mul(nrt[:, b0:b1], rr[:, b0:b1], rr[:, b0:b1])
                        nc.vector.tensor_mul(nrt[:, b0:b1], nrt[:, b0:b1], ssq[:, b0:b1])
                        nc.vector.tensor_scalar(
                            nrt[:, b0:b1], nrt[:, b0:b1], -0.5, 1.5, OP.mult, OP.add
                        )
                        nc.vector.tensor_mul(rr[:, b0:b1], rr[:, b0:b1], nrt[:, b0:b1])

            # ---- phase 3: normalized transpose via PE matmul with diag(r).
            # out[d, a] = sum_rows feats[row, d] * (eye[row, a] * r_row)
            #           = feats[a, d] * r_a
            for u in range(W if stop_after >= 3 else 0):
                if u < TS:
                    src = slab_sb[:, u, :]
                else:
                    src = feats_sb[:, u - TS, :]
                dvr = wp.tile([P, P], f32, tag="dvr")
                nc.vector.tensor_scalar_mul(dvr[:], eye[:], rr[:, u : u + 1])
                for k in range(2):
                    ps = ptp.tile([P, P], f32)
                    nc.tensor.matmul(
                        ps[:],
                        src[:, k * P : (k + 1) * P],
                        dvr[:],
                        start=True,
                        stop=True,
                    )
                    if u < TS:
                        dst32 = (fTs32_0, fTs32_1)[k]
                        dst16 = (fTs16_0, fTs16_1)[k]
                        nc.vector.tensor_copy(
                            dst32[:, u * P : (u + 1) * P], ps[:]
                        )
                        nc.vector.tensor_copy(
                            dst16[:, u * P : (u + 1) * P], ps[:]
                        )
                    else:
                        t = u - TS
                        dstb = (fTb_0, fTb_1)[k]
                        nc.vector.tensor_copy(dstb[:, t * P : (t + 1) * P], ps[:])

            # ---- phase 4: main bf16 sweep with fused exp + row-sum
            for m in range(TS if stop_after >= 4 else 0):
                msl = slice(m * P, (m + 1) * P)
                for n in range(NCHUNKS):
                    nsl = slice(n * NCH, (n + 1) * NCH)
                    ps = pmm.tile([P, NCH], f32)
                    nc.tensor.matmul(
                        ps[:], fTs16_0[:, msl], fTb_0[:, nsl], start=True, stop=False
                    )
                    nc.tensor.matmul(
                        ps[:], fTs16_1[:, msl], fTb_1[:, nsl], start=False, stop=True
                    )
                    eo = wp.tile([P, NCH], bf16, tag="eo")
                    col = m * NCHUNKS + n
                    nc.scalar.activation(
                        eo[:],
                        ps[:],
                        EXP,
                        scale=INV_TAU,
                        accum_out=accs[:, col : col + 1],
                    )

            # ---- phase 5: fp32 diagonal blocks + bf16 recompute + masked sums
            for m in range(TS if stop_after >= 5 else 0):
                msl = slice(m * P, (m + 1) * P)
                ps32 = pdg.tile([P, P], f32, tag="dgps")
                nc.tensor.matmul(
                    ps32[:], fTs32_0[:, msl], fTs32_0[:, msl], start=True, stop=False
                )
                nc.tensor.matmul(
                    ps32[:], fTs32_1[:, msl], fTs32_1[:, msl], start=False, stop=True
                )
                E32 = wp.tile([P, P], f32, tag="E32")
                nc.scalar.activation(
                    E32[:], ps32[:], EXP, scale=INV_TAU, accum_out=B32[:, m : m + 1]
                )

                psb = pdg.tile([P, P], f32, tag="dgps")
                nc.tensor.matmul(
                    psb[:], fTs16_0[:, msl], fTs16_0[:, msl], start=True, stop=False
                )
                nc.tensor.matmul(
                    psb[:], fTs16_1[:, msl], fTs16_1[:, msl], start=False, stop=True
                )
                eob = wp.tile([P, P], bf16, tag="eob")
                nc.scalar.activation(
                    eob[:], psb[:], EXP, scale=INV_TAU, accum_out=B16[:, m : m + 1]
                )

                junk1 = wp.tile([P, P], f32, tag="ttr_junk1")
                nc.vector.tensor_mul(junk1[:], E32[:], posmask[:])
                nc.vector.tensor_reduce(
                    out=POS[:, m : m + 1],
                    in_=junk1[:],
                    axis=mybir.AxisListType.X,
                    op=OP.add,
                )
                junk2 = wp.tile([P, P], f32, tag="ttr_junk2")
                nc.vector.tensor_mul(junk2[:], E32[:], eye[:])
                nc.vector.tensor_reduce(
                    out=EII[:, m : m + 1],
                    in_=junk2[:],
                    axis=mybir.AxisListType.X,
                    op=OP.add,
                )

            # ---- phase 6: combine and store
            if stop_after >= 6:
                for m in range(TS):
                    nc.vector.tensor_reduce(
                        out=Racc[:, m : m + 1],
                        in_=accs[:, m * NCHUNKS : (m + 1) * NCHUNKS],
                        axis=mybir.AxisListType.X,
                        op=OP.add,
                    )
                nc.vector.tensor_sub(NEG[:], Racc[:], B16[:])
                nc.vector.tensor_add(NEG[:], NEG[:], B32[:])
                nc.vector.tensor_sub(NEG[:], NEG[:], POS[:])
                nc.vector.tensor_sub(NEG[:], NEG[:], EII[:])

                nc.sync.dma_start(pos_d.ap().rearrange("(m p) -> p m", p=P), POS[:])
                nc.sync.dma_start(neg_d.ap().rearrange("(m p) -> p m", p=P), NEG[:])
            else:
                _finish_early()

    nc.compile()
    return nc


def _masks_from_num_crops(num_crops: np.ndarray):
    nca = np.asarray(num_crops).astype(np.int64)
    assert int(nca.sum()) == N, f"num_crops sums to {nca.sum()}, expected {N}"
    assert np.all(nca == GROUP), "kernel specialized for constant group size 8"
    blk = np.ones((GROUP, GROUP), dtype=np.float32)
    full = np.kron(np.eye(P // GROUP, dtype=np.float32), blk)
    eye = np.eye(P, dtype=np.float32)
    posmask = full - eye
    return posmask, eye


def _get_program():
    if "nc" not in _cache:
        _cache["nc"] = _build(debug=False)
    return _cache["nc"]


def _run(features: np.ndarray, num_crops: np.ndarray, **spmd_kwargs):
    feats = np.ascontiguousarray(np.asarray(features, dtype=np.float32))
    assert feats.shape == (N, D)
    posmask, eye = _masks_from_num_crops(num_crops)

    nc = _get_program()
    in_maps = [
        {
            "feats": feats,
            "slab": np.ascontiguousarray(feats[c * SLAB : (c + 1) * SLAB]),
            "posmask": posmask,
            "eyemask": eye,
        }
        for c in range(CORES)
    ]
    br = run_bass_kernel_spmd(nc, in_maps, list(range(CORES)), **spmd_kwargs)
    res = br.results
    pos = np.concatenate([res[c]["pos"] for c in range(CORES)]).astype(np.float64)
    neg = np.concatenate([res[c]["neg"] for c in range(CORES)]).astype(np.float64)
    loss = np.mean(np.log(neg) - np.log(pos))
    return np.asarray(loss, dtype=np.float32), br


def kernel(features: np.ndarray, num_crops: np.ndarray) -> np.ndarray:
    loss, _ = _run(features, num_crops)
    return loss
